# revision 1
# baseline (speedup 1.0000x reference)
"""Trainium2 Bass kernel for nn_MultiHeadAttention_78237124264578.

Reference computation (NO softmax — attention is purely bilinear):
    q = (x @ Wq.T + bq).reshape(8, 2, 2048, 64)   # FLAT reshape
    att = einsum('hbid,hbjd->hbij', q, k) * 64**-0.5
    out = einsum('hbij,hbjd->hbid', att, v)
    return out.transpose(1,2,3,0).reshape(2, 2048, 512)

Key identities exploited:
  1. (q kT) v == q (kT v): the 2048x2048 attention matrix collapses to a
     64x64 Gram matrix S = K^T V per (head, block).
  2. The head reshape is flat: head h / block b2 of Q/K/V is just rows
     [512h + 256 b2, 512h + 256(b2+1)) of the [4096, 512] projection
     output, reinterpreted [256,512]->[2048,64].  So core i only needs
     x rows [512i, 512(i+1)) plus the full (512x512) weights.

Sharding: head i -> core i (tensor parallel over nhead; both b2 blocks
of a head live on the same core).  Inputs are pre-transposed on the
host so every matmul contraction lands on the partition dim:
    xT_i  = x_flat[512i:512(i+1)].T          [512(k), 512(r)]
    W*T   = W*.T                              [512(k), 512(f)]
Per core the device computes:
    YqT[f,r] = sum_k WqT[k,f] xT[k,r]   (transposed layout; per-partition
               bias add + 0.125 scale folded into the PSUM->SBUF copy)
    Yk[r,f]  = sum_k xT[k,r] WkT[k,f] + bk   (bias broadcast on GpSimd)
    Yv[r,f]  likewise
    per b2:  S[d1,d2] = sum_{r,f_hi} Yk[r, f_hi*64+d1] Yv[r, f_hi*64+d2]
             OT[f_hi*64+d, r] = sum_d1 S[d1,d] YqT[f_hi*64+d1, r]
Output "ot" [512, 512] = OT; host stacks heads and untangles layout.

All matmuls run in float32r (TF32-like: RNE to 11 mantissa bits,
~1.5e-4 rel err per matmul, 4x faster than float32 on the PE).  The
x/W DRAM+SBUF tensors are declared float32r so the DMA feeds matmuls
directly — the PE rounds operands on ingest; no rounding copies needed.
"""

import functools

import numpy as np

NCORES = 8
NIN = 512          # input features = contraction dim
NF = 512           # projection output features
R = 512            # rows per core (one head)
KC = NIN // 128    # contraction chunks
FC = NF // 128     # feature/row chunks
DIM = 64
SCALE = DIM ** -0.5

# Tiny fp32 warm-up matmuls issued before the real work (ramps the PE
# clock while the first input DMAs are in flight).
N_WARMUP = 6


@functools.lru_cache(maxsize=1)
def _build():
    from concourse import bacc
    import concourse.mybir as mybir
    import concourse.tile as tile

    f32 = mybir.dt.float32
    f32r = mybir.dt.float32r

    nc = bacc.Bacc(None, target_bir_lowering=False)

    xt_d = nc.dram_tensor("xt", [NIN, R], f32r, kind="ExternalInput")
    wqt_d = nc.dram_tensor("wqt", [NIN, NF], f32r, kind="ExternalInput")
    wkt_d = nc.dram_tensor("wkt", [NIN, NF], f32r, kind="ExternalInput")
    wvt_d = nc.dram_tensor("wvt", [NIN, NF], f32r, kind="ExternalInput")
    bqc_d = nc.dram_tensor("bqc", [128, FC], f32, kind="ExternalInput")  # 0.125*bq, [p,c]
    brow_d = nc.dram_tensor("brow", [1, 2 * NF], f32, kind="ExternalInput")  # bk|bv
    ot_d = nc.dram_tensor("ot", [NF, R], f32, kind="ExternalOutput")

    with tile.TileContext(nc) as tc:
        with (
            tc.tile_pool(name="sb", bufs=1) as sb,
            tc.tile_pool(name="pacc", bufs=4, space="PSUM") as pacc,
            tc.tile_pool(name="pso", bufs=4, space="PSUM") as pso,
        ):
            # ---- PE warm-up (bridges until the first operands land) --------
            wu = sb.tile([1, 128], f32, tag="wu", name="wu")
            nc.gpsimd.memset(wu[:], 0.0)
            for i in range(N_WARMUP):
                psw = pso.tile([1, 128], f32, tag="o", name=f"psw{i}")
                nc.tensor.matmul(psw[:], wu[0:1, 0:1], wu[:])

            # ---- input DMAs: 2 per tensor (HWDGE issue cost dominates
            # small transfers), biases on the SWDGE path (GpSimd)  ----------
            # each tile holds 2 contraction chunks: [128, 2, 512]
            xt_r = [sb.tile([128, 2, R], f32r, tag=f"xtr{t}", name=f"xtr{t}") for t in range(2)]
            wq_r = [sb.tile([128, 2, NF], f32r, tag=f"wqr{t}", name=f"wqr{t}") for t in range(2)]
            wk_r = [sb.tile([128, 2, NF], f32r, tag=f"wkr{t}", name=f"wkr{t}") for t in range(2)]
            wv_r = [sb.tile([128, 2, NF], f32r, tag=f"wvr{t}", name=f"wvr{t}") for t in range(2)]

            def op(tiles, k):
                return tiles[k // 2][:, k % 2, :]

            def opm(tiles, k, c):
                return tiles[k // 2][:, k % 2, 128 * c:128 * (c + 1)]

            for t in range(2):
                sl = slice(256 * t, 256 * (t + 1))
                nc.sync.dma_start(
                    xt_r[t][:], xt_d[sl, :].rearrange("(c p) r -> p c r", p=128))
                nc.sync.dma_start(
                    wk_r[t][:], wkt_d[sl, :].rearrange("(c p) f -> p c f", p=128))
            for t in range(2):
                sl = slice(256 * t, 256 * (t + 1))
                nc.sync.dma_start(
                    wv_r[t][:], wvt_d[sl, :].rearrange("(c p) f -> p c f", p=128))
            for t in range(2):
                sl = slice(256 * t, 256 * (t + 1))
                nc.sync.dma_start(
                    wq_r[t][:], wqt_d[sl, :].rearrange("(c p) f -> p c f", p=128))

            bqc = sb.tile([128, FC], f32, tag="bqc")
            brow = sb.tile([1, 2 * NF], f32, tag="brow")
            bkb = sb.tile([128, NF], f32, tag="bkb")
            bvb = sb.tile([128, NF], f32, tag="bvb")
            nc.gpsimd.dma_start(brow[:], brow_d[:, :])
            nc.gpsimd.dma_start(bqc[:], bqc_d[:, :])
            nc.gpsimd.partition_broadcast(bkb[:], brow[0:1, 0:NF])
            nc.gpsimd.partition_broadcast(bvb[:], brow[0:1, NF:2 * NF])

            q_sb = [sb.tile([128, R], f32r, tag=f"q{c}", name=f"q{c}") for c in range(FC)]
            k_sb = [sb.tile([128, NF], f32r, tag=f"k{c}", name=f"k{c}") for c in range(FC)]
            v_sb = [sb.tile([128, NF], f32r, tag=f"v{c}", name=f"v{c}") for c in range(FC)]

            # ---- Yk chunks: out[r-chunk, f] --------------------------------
            psk = [pacc.tile([128, NF], f32, tag="acc", name=f"psk{c}") for c in range(FC)]
            for k in range(KC):
                for c in range(FC):
                    nc.tensor.matmul(
                        psk[c][:], opm(xt_r, k, c), op(wk_r, k),
                        start=(k == 0), stop=(k == KC - 1),
                    )

            # ---- Yv chunks -------------------------------------------------
            psv = [pacc.tile([128, NF], f32, tag="acc", name=f"psv{c}") for c in range(FC)]
            for k in range(KC):
                for c in range(FC):
                    nc.tensor.matmul(
                        psv[c][:], opm(xt_r, k, c), op(wv_r, k),
                        start=(k == 0), stop=(k == KC - 1),
                    )
            # bias adds, ordered so chunks 0/1 (needed by S of b2=0) retire
            # first on the DVE
            for c in (0, 1):
                nc.vector.tensor_add(k_sb[c][:], psk[c][:], bkb[:])
                nc.vector.tensor_add(v_sb[c][:], psv[c][:], bvb[:])
            for c in (2, 3):
                nc.vector.tensor_add(k_sb[c][:], psk[c][:], bkb[:])
                nc.vector.tensor_add(v_sb[c][:], psv[c][:], bvb[:])

            # ---- YqT chunks: out[f-chunk, r]; Q needed only by the O
            # phase, so it runs after S (its weights also arrive last).
            # bias+scale folded into the PSUM->SBUF copy (ACT/DVE split).
            psq = [pacc.tile([128, R], f32, tag="acc", name=f"psq{c}") for c in range(FC)]
            for k in range(KC):
                for c in range(FC):
                    nc.tensor.matmul(
                        psq[c][:], opm(wq_r, k, c), op(xt_r, k),
                        start=(k == 0), stop=(k == KC - 1),
                    )
            for c in range(FC):
                if c % 2 == 0:
                    nc.scalar.activation(
                        q_sb[c][:], psq[c][:],
                        mybir.ActivationFunctionType.Identity,
                        bias=bqc[:, c:c + 1], scale=SCALE,
                    )
                else:
                    nc.vector.tensor_scalar(
                        q_sb[c][:], psq[c][:], SCALE, bqc[:, c:c + 1],
                        mybir.AluOpType.mult, mybir.AluOpType.add,
                    )

            # ---- attention: S = K^T V, OT = S^T Q^T ------------------------
            # S lives in SBUF twice (partitions 0:64 and 64:128) because a
            # matmul requires lhsT and rhs at the same base partition, and
            # the odd-f_hi Q blocks sit at partition base 64.  Matmul PSUM
            # dst must always be base 0 (walrus s3d3_mm_valid_dst_partition);
            # engine copies handle the partition shifts.
            for b2 in range(2):
                ps_s = pacc.tile([64, 64], f32, tag="acc", name=f"ps_s{b2}")
                idx = 0
                for rc in (2 * b2, 2 * b2 + 1):
                    for fh in range(8):
                        nc.tensor.matmul(
                            ps_s[:],
                            k_sb[rc][:, 64 * fh:64 * (fh + 1)],
                            v_sb[rc][:, 64 * fh:64 * (fh + 1)],
                            start=(idx == 0), stop=(idx == 15),
                        )
                        idx += 1
                s2 = sb.tile([128, 64], f32r, tag=f"s{b2}", name=f"s2_{b2}")
                nc.vector.tensor_copy(s2[0:64, :], ps_s[:])
                nc.vector.tensor_copy(s2[64:128, :], ps_s[:])  # shifted dup

                for c in range(FC):
                    rsl = slice(256 * b2, 256 * (b2 + 1))
                    ps_oe = pso.tile([64, 256], f32, tag="o", name=f"ps_oe{b2}_{c}")
                    ps_oo = pso.tile([64, 256], f32, tag="o", name=f"ps_oo{b2}_{c}")
                    nc.tensor.matmul(ps_oe[:], s2[0:64, :], q_sb[c][0:64, rsl])
                    nc.tensor.matmul(ps_oo[:], s2[64:128, :], q_sb[c][64:128, rsl])
                    ot = sb.tile([128, 256], f32, tag=f"ot{b2}_{c}", name=f"ot{b2}_{c}")
                    nc.vector.tensor_copy(ot[0:64, :], ps_oe[:])
                    nc.scalar.copy(ot[64:128, :], ps_oo[:])
                    nc.sync.dma_start(ot_d[128 * c:128 * (c + 1), rsl], ot[:])

    nc.compile()
    return nc


def kernel(x, Wq, bq, Wk, bk, Wv, bv):
    from concourse.bass_utils import run_bass_kernel_spmd

    x = np.asarray(x, dtype=np.float32)
    Wq = np.asarray(Wq, dtype=np.float32)
    Wk = np.asarray(Wk, dtype=np.float32)
    Wv = np.asarray(Wv, dtype=np.float32)
    bq = np.asarray(bq, dtype=np.float32)
    bk = np.asarray(bk, dtype=np.float32)
    bv = np.asarray(bv, dtype=np.float32)

    B, N, nin = x.shape
    x_flat = x.reshape(B * N, nin)                       # [4096, 512]

    wqt = np.ascontiguousarray(Wq.T)
    wkt = np.ascontiguousarray(Wk.T)
    wvt = np.ascontiguousarray(Wv.T)
    bqc = np.ascontiguousarray((SCALE * bq).reshape(FC, 128).T)  # [p, c]
    brow = np.ascontiguousarray(
        np.concatenate([bk, bv]).reshape(1, 2 * NF))

    in_maps = []
    for i in range(NCORES):
        xt_i = np.ascontiguousarray(x_flat[R * i:R * (i + 1)].T)
        in_maps.append({
            "xt": xt_i, "wqt": wqt, "wkt": wkt, "wvt": wvt,
            "bqc": bqc, "brow": brow,
        })

    nc = _build()
    res = run_bass_kernel_spmd(nc, in_maps, core_ids=list(range(NCORES)))

    # ot[i][f_hi*64+d, b2*256+rr] = out[h=i, b2, n2=rr*8+f_hi, d]
    ot = np.stack([res.results[i]["ot"] for i in range(NCORES)])  # [h, f', r]
    ot = ot.reshape(NCORES, 8, DIM, 2, 256)                       # [h, fh, d, b2, rr]
    z = ot.transpose(3, 4, 1, 2, 0).reshape(B, N, 8 * DIM)        # [b2, n2, d*8+h]
    return np.ascontiguousarray(z)



# revision 8
# speedup vs baseline: 1.2824x; 1.2824x over previous
"""Trainium2 Bass kernel for nn_MultiHeadAttention_78237124264578.

Reference computation (NO softmax — attention is purely bilinear):
    q = (x @ Wq.T + bq).reshape(8, 2, 2048, 64)   # FLAT reshape
    att = einsum('hbid,hbjd->hbij', q, k) * 64**-0.5
    out = einsum('hbij,hbjd->hbid', att, v)
    return out.transpose(1,2,3,0).reshape(2, 2048, 512)

Key identities exploited:
  1. (q kT) v == q (kT v): the 2048x2048 attention matrix collapses to a
     64x64 Gram matrix S = K^T V per (head, block b2).
  2. The head reshape is flat: head h / block b2 of Q/K/V is rows
     [512h + 256 b2, 512h + 256(b2+1)) of the [4096, 512] projection
     output, reinterpreted [256,512]->[2048,64].  So core i only needs
     x rows [512i, 512(i+1)) plus the full (512x512) weights.
  3. O_chunk[256r, 512f] = Yq_chunk @ (I8 (x) S): per 128-col chunk c the
     transposed output OT[128, 512r] = blockdiag(S, S)^T @ YqT_chunk, so
     one 128-partition matmul per (b2, chunk) computes O.

Sharding: head i -> core i.  All inputs shipped bf16 (halves the DMA,
matmul rate identical to fp32r, and small-free-size matmuls avoid the
fp32r 4x penalty).  The 0.125 attention scale is folded into Wq/bq on
the host.  Inputs are packed into ONE dram tensor in consumption order
so 8 pipelined DMAs feed the PE without stalls:
    slots: xt0 wk0 xt1 wk1 xt2 wk2 xt3 wk3 | wv0..3 | wq0..3
Stores go straight from PSUM to DRAM (no SBUF bounce), one [128,512]
DMA per output chunk.
"""

import functools

import numpy as np

NCORES = 8
NIN = 512          # input features = contraction dim
NF = 512           # projection output features
R = 512            # rows per core (one head)
KC = NIN // 128    # contraction chunks
FC = NF // 128     # feature/row chunks
DIM = 64
SCALE = DIM ** -0.5


@functools.lru_cache(maxsize=1)
def _build():
    from concourse import bacc
    import concourse.mybir as mybir
    import concourse.tile as tile

    f32 = mybir.dt.float32
    bf16 = mybir.dt.bfloat16

    nc = bacc.Bacc(None, target_bir_lowering=False)

    # packed operands: 16 slots of [128, 512] bf16, consumption order
    inp_d = nc.dram_tensor("inp", [128, 16, 512], bf16, kind="ExternalInput")
    brow_d = nc.dram_tensor("brow", [1, 2 * NF], f32, kind="ExternalInput")  # bk|bv
    bqc_d = nc.dram_tensor("bqc", [128, FC], f32, kind="ExternalInput")  # 0.125*bq
    ot_d = nc.dram_tensor("ot", [NF, R], bf16, kind="ExternalOutput")

    XT = [0, 2, 4, 6]   # xt k-chunk slots
    WK = [1, 3, 5, 7]
    WV = [8, 9, 10, 11]
    WQ = [12, 13, 14, 15]

    with tile.TileContext(nc) as tc:
        with (
            tc.tile_pool(name="sb", bufs=1) as sb,
            tc.tile_pool(name="pa", bufs=4, space="PSUM") as pa,
            tc.tile_pool(name="pb", bufs=4, space="PSUM") as pb,
        ):
            # ---- PE warm-up: DVE memset so the PE can start ~t=0 ----------
            wu = sb.tile([1, 64], f32, tag="wu", name="wu")
            nc.vector.memset(wu[:], 0.0)
            for i in range(4):
                psw = pb.tile([1, 64], f32, tag="B", name=f"psw{i}")
                nc.tensor.matmul(psw[:], wu[0:1, 0:1], wu[:])

            # ---- input DMAs: first (xt0|wk0) via Pool/SWDGE, rest HWDGE ----
            ops = sb.tile([128, 16, 512], bf16, tag="ops", name="ops")
            nc.gpsimd.dma_start(ops[:, 0:2, :], inp_d[:, 0:2, :])
            for t in range(1, 8):
                nc.sync.dma_start(
                    ops[:, 2 * t:2 * t + 2, :], inp_d[:, 2 * t:2 * t + 2, :])

            brow = sb.tile([1, 2 * NF], f32, tag="brow")
            bqc = sb.tile([128, FC], f32, tag="bqc")
            bkb = sb.tile([128, NF], f32, tag="bkb")
            bvb = sb.tile([128, NF], f32, tag="bvb")
            nc.gpsimd.dma_start(brow[:], brow_d[:, :])
            nc.gpsimd.dma_start(bqc[:], bqc_d[:, :])
            nc.gpsimd.partition_broadcast(bkb[:], brow[0:1, 0:NF])
            nc.gpsimd.partition_broadcast(bvb[:], brow[0:1, NF:2 * NF])

            # blockdiag(S,S) operand tiles (off-diagonal stays zero)
            s2b = [sb.tile([128, 128], bf16, tag=f"s2b{b}", name=f"s2b{b}")
                   for b in range(2)]
            nc.gpsimd.memset(s2b[0][:], 0.0)
            nc.gpsimd.memset(s2b[1][:], 0.0)

            k_sb = [sb.tile([128, NF], bf16, tag=f"k{c}", name=f"k{c}") for c in range(FC)]
            v_sb = [sb.tile([128, NF], bf16, tag=f"v{c}", name=f"v{c}") for c in range(FC)]
            q_sb = [sb.tile([128, R], bf16, tag=f"q{c}", name=f"q{c}") for c in range(FC)]

            def slot(s):
                return ops[:, s, :]

            def slotc(s, c):
                return ops[:, s, 128 * c:128 * (c + 1)]

            # ---- Yk: psk[c][r,f], k-outer to match DMA arrival -------------
            psk = [pa.tile([128, NF], f32, tag="A", name=f"psk{c}") for c in range(FC)]
            for k in range(KC):
                for c in range(FC):
                    nc.tensor.matmul(
                        psk[c][:], slotc(XT[k], c), slot(WK[k]),
                        start=(k == 0), stop=(k == KC - 1),
                    )
            # K bias adds: PSUM tensor+tensor is DVE-only; c0/c1 first (S0)
            nc.vector.tensor_add(k_sb[0][:], psk[0][:], bkb[:])
            nc.vector.tensor_add(k_sb[1][:], psk[1][:], bkb[:])

            psv = [pb.tile([128, NF], f32, tag="B", name=f"psv{c}") for c in range(FC)]

            def yv(c):
                for k in range(KC):
                    nc.tensor.matmul(
                        psv[c][:], slotc(XT[k], c), slot(WV[k]),
                        start=(k == 0), stop=(k == KC - 1),
                    )

            def vbias(c):
                nc.vector.tensor_add(v_sb[c][:], psv[c][:], bvb[:])

            psq = [pa.tile([128, R], f32, tag="A", name=f"psq{c}") for c in range(FC)]

            def yq(c, half=None):
                sl = slice(None) if half is None else slice(256 * half, 256 * (half + 1))
                for k in range(KC):
                    nc.tensor.matmul(
                        psq[c][:, sl], slotc(WQ[k], c), slot(XT[k])[:, sl],
                        start=(k == 0), stop=(k == KC - 1),
                    )

            def s_mm(ps_s, b2):
                idx = 0
                for rc in (2 * b2, 2 * b2 + 1):
                    for fh in range(8):
                        nc.tensor.matmul(
                            ps_s[:],
                            k_sb[rc][:, 64 * fh:64 * (fh + 1)],
                            v_sb[rc][:, 64 * fh:64 * (fh + 1)],
                            start=(idx == 0), stop=(idx == 15),
                        )
                        idx += 1

            # ---- pipeline: Yv c0/c1 -> Yq c0 -> S0 -> Yv c2/c3 -> Yq c1 ->
            #      S1 -> Yq c2 -> O pairs + stores -> Yq c3 (split) ---------
            def qbias_act(c, sl=slice(None)):
                nc.scalar.activation(
                    q_sb[c][:, sl], psq[c][:, sl],
                    mybir.ActivationFunctionType.Identity,
                    bias=bqc[:, c:c + 1], scale=1.0,
                )

            yv(0)
            yv(1)
            vbias(0)
            vbias(1)
            yq(0)
            qbias_act(0)
            ps_s0 = pb.tile([64, 64], f32, tag="B", name="ps_s0")
            s_mm(ps_s0, 0)
            # K c2/c3 adds queue on DVE behind v0/v1; blockdiag halves
            # split DVE/ACT for latency
            nc.vector.tensor_add(k_sb[2][:], psk[2][:], bkb[:])
            nc.vector.tensor_add(k_sb[3][:], psk[3][:], bkb[:])
            nc.vector.tensor_copy(s2b[0][0:64, 0:64], ps_s0[:])
            nc.scalar.copy(s2b[0][64:128, 64:128], ps_s0[:])
            yv(2)
            vbias(2)
            yv(3)
            vbias(3)
            yq(1)
            qbias_act(1)
            ps_s1 = pb.tile([64, 64], f32, tag="B", name="ps_s1")
            s_mm(ps_s1, 1)
            nc.vector.tensor_copy(s2b[1][0:64, 0:64], ps_s1[:])
            nc.scalar.copy(s2b[1][64:128, 64:128], ps_s1[:])
            yq(2)
            qbias_act(2)

            # output chunks: ps_oc[c] [128, 512], col half b2 from s2b[b2]
            ps_oc = [None] * FC
            ps_oc[0] = pb.tile([128, R], f32, tag="B", name="ps_oc0")
            ps_oc[1] = pb.tile([128, R], f32, tag="B", name="ps_oc1")
            ps_oc[2] = pa.tile([128, R], f32, tag="A", name="ps_oc2")
            ps_oc[3] = pa.tile([128, R], f32, tag="A", name="ps_oc3")

            oc_sb = [sb.tile([128, R], bf16, tag=f"oc{c}", name=f"oc{c}")
                     for c in range(FC)]

            def o_pair(c):
                for b2 in range(2):
                    rsl = slice(256 * b2, 256 * (b2 + 1))
                    nc.tensor.matmul(ps_oc[c][:, rsl], s2b[b2][:], q_sb[c][:, rsl])
                # PSUM -> SBUF bf16, halves split DVE/ACT for latency
                nc.vector.tensor_copy(oc_sb[c][:, 0:256], ps_oc[c][:, 0:256])
                nc.scalar.copy(oc_sb[c][:, 256:512], ps_oc[c][:, 256:512])
                nc.sync.dma_start(ot_d[128 * c:128 * (c + 1), :], oc_sb[c][:])

            o_pair(0)
            o_pair(1)
            yq(3, half=0)
            nc.vector.tensor_scalar_add(
                q_sb[3][:, 0:256], psq[3][:, 0:256], bqc[:, 3:4])
            yq(3, half=1)
            qbias_act(3, sl=slice(256, 512))
            o_pair(2)
            o_pair(3)

    nc.compile()
    return nc


def kernel(x, Wq, bq, Wk, bk, Wv, bv):
    import ml_dtypes
    from concourse.bass_utils import run_bass_kernel_spmd

    bf16 = ml_dtypes.bfloat16
    x = np.asarray(x, dtype=np.float32)
    Wq = np.asarray(Wq, dtype=np.float32)
    Wk = np.asarray(Wk, dtype=np.float32)
    Wv = np.asarray(Wv, dtype=np.float32)
    bq = np.asarray(bq, dtype=np.float32)
    bk = np.asarray(bk, dtype=np.float32)
    bv = np.asarray(bv, dtype=np.float32)

    B, N, nin = x.shape
    x_flat = x.reshape(B * N, nin)                       # [4096, 512]

    wkt = Wk.T.astype(bf16)                              # [k, f]
    wvt = Wv.T.astype(bf16)
    wqt = (SCALE * Wq).T.astype(bf16)
    brow = np.ascontiguousarray(
        np.concatenate([bk, bv]).reshape(1, 2 * NF))
    bqc = np.ascontiguousarray((SCALE * bq).reshape(FC, 128).T)  # [p, c]

    def chunks(t):
        return [t[128 * j:128 * (j + 1)] for j in range(4)]

    wk_c, wv_c, wq_c = chunks(wkt), chunks(wvt), chunks(wqt)

    in_maps = []
    for i in range(NCORES):
        xt_i = x_flat[R * i:R * (i + 1)].T.astype(bf16)  # [k, r]
        xt_c = chunks(xt_i)
        slots = [xt_c[0], wk_c[0], xt_c[1], wk_c[1],
                 xt_c[2], wk_c[2], xt_c[3], wk_c[3],
                 *wv_c, *wq_c]
        inp = np.ascontiguousarray(np.stack(slots, axis=1))  # [128, 16, 512]
        in_maps.append({"inp": inp, "brow": brow, "bqc": bqc})

    nc = _build()
    res = run_bass_kernel_spmd(nc, in_maps, core_ids=list(range(NCORES)))

    # ot[i][f_hi*64+d, b2*256+rr] = out[h=i, b2, n2=rr*8+f_hi, d]
    ot = np.stack([np.asarray(res.results[i]["ot"], dtype=np.float32)
                   for i in range(NCORES)])                       # [h, f', r]
    ot = ot.reshape(NCORES, 8, DIM, 2, 256)                       # [h, fh, d, b2, rr]
    z = ot.transpose(3, 4, 1, 2, 0).reshape(B, N, 8 * DIM)        # [b2, n2, d*8+h]
    return np.ascontiguousarray(z)
